# revision 9
# baseline (speedup 1.0000x reference)
"""Fused multi-head causal attention (RoPE) for Trainium2, 8-core SPMD.

Sharding: data-parallel over batch (B=2) x tensor-parallel over heads
(16 heads -> 4 per core, Megatron-style column/row split of the qkv/out
projections). Each core computes a partial (T, C) output; the host sums
the 4 partials per batch element.

Per-core layout:
  - x pre-transposed on host (xT: C x T) so every matmul contraction has
    its contracted dim on SBUF partitions; bf16 operands everywhere on
    the PE (1 cyc/row + fast weight load), fp32 PSUM accumulation.
  - QKV produces q,k transposed [feat, tok] and v natural [tok, feat];
    RoPE runs in [feat, tok] layout, the half-rotation partition swap is
    done by SBUF->SBUF DMA.
  - Scores are computed transposed S^T[k, q] so the softmax sum over k
    becomes a matmul contraction (ones column appended to v); exp runs
    on ScalarE straight out of PSUM with the 1/sqrt(D) folded into the
    activation scale. No max-subtraction (scores are ~N(0,1)). Causal
    masking adds -1e9 on diagonal tiles via an identity matmul, and all
    diagonal-tile matmuls are narrowed to the live column range.
"""

import sys
import numpy as np

if '/opt/trn_rl_repo' not in sys.path:
    sys.path.insert(0, '/opt/trn_rl_repo')

import ml_dtypes

B, T, C, H, D = 2, 2048, 1024, 16, 64
HPC = 4            # heads per core
NCORES = 8
NEG = -1.0e9
BF = ml_dtypes.bfloat16

_cache = {}


def _build():
    import concourse.mybir as mybir
    from concourse import bacc
    import concourse.tile as tile

    F32 = mybir.dt.float32
    FR = mybir.dt.float32r
    B16 = mybir.dt.bfloat16
    AF = mybir.ActivationFunctionType

    nc = bacc.Bacc("TRN2", debug=False, enable_asserts=True)
    xT = nc.dram_tensor("xT", [C, T], B16, kind="ExternalInput").ap()
    wqk = nc.dram_tensor("wqk", [C, 512], B16, kind="ExternalInput").ap()
    wv = nc.dram_tensor("wv", [C, 256], B16, kind="ExternalInput").ap()
    wo = nc.dram_tensor("wo", [256, 1024], B16, kind="ExternalInput").ap()
    cosR = nc.dram_tensor("cosR", [128, T], B16, kind="ExternalInput").ap()
    sinS = nc.dram_tensor("sinS", [128, T], B16, kind="ExternalInput").ap()
    maskM = nc.dram_tensor("maskM", [128, 2048], B16, kind="ExternalInput").ap()
    ident = nc.dram_tensor("ident", [128, 128], B16, kind="ExternalInput").ap()
    onesI = nc.dram_tensor("onesI", [128, 64], FR, kind="ExternalInput").ap()
    out = nc.dram_tensor("out", [T, C], F32, kind="ExternalOutput").ap()

    NQ = 4            # token quarters for streaming x
    QT = T // NQ      # 512 tokens per quarter

    with tile.TileContext(nc) as tc:
        with tc.tile_pool(name="persist", bufs=1) as pp, \
             tc.tile_pool(name="rawp", bufs=2) as rawp, \
             tc.tile_pool(name="swpp", bufs=2) as swpp, \
             tc.tile_pool(name="ptp", bufs=2) as ptp, \
             tc.tile_pool(name="stagep", bufs=2) as stagep, \
             tc.tile_pool(name="nrmp", bufs=2) as nrmp, \
             tc.tile_pool(name="ysbp", bufs=2) as ysbp, \
             tc.tile_pool(name="psA", bufs=2, space="PSUM") as psA, \
             tc.tile_pool(name="psO", bufs=2, space="PSUM") as psO:

            # ---- persistent SBUF tensors ----
            wqk_sb = pp.tile([128, 8 * 512], B16, tag="wqk")
            wv_sb = pp.tile([128, 8 * 256], B16, tag="wv")
            wo_sb = pp.tile([128, 2 * 1024], B16, tag="wo")
            cos_sb = pp.tile([128, T], B16, tag="cos")
            sin_sb = pp.tile([128, T], B16, tag="sin")
            mask_sb = pp.tile([128, 2048], B16, tag="mask")
            id_sb = pp.tile([128, 128], B16, tag="id")
            ones_sb = pp.tile([128, 64], FR, tag="ones")
            onesb_sb = pp.tile([128, 64], B16, tag="onesb")
            qk_rot = [pp.tile([128, T], B16, tag=f"rot{i}", name=f"rot{i}") for i in range(4)]
            v_sb = pp.tile([128, 16 * HPC * 65], B16, tag="v")
            aou = [pp.tile([128, T], B16, tag=f"aou{i}", name=f"aou{i}") for i in range(2)]
            xq_sb = pp.tile([128, 8 * QT], B16, tag="xq")  # one token-quarter of xT

            for k in range(8):
                nc.sync.dma_start(wqk_sb[:, k * 512:(k + 1) * 512], wqk[k * 128:(k + 1) * 128, :])
                nc.sync.dma_start(wv_sb[:, k * 256:(k + 1) * 256], wv[k * 128:(k + 1) * 128, :])
            for k in range(2):
                nc.sync.dma_start(wo_sb[:, k * 1024:(k + 1) * 1024], wo[k * 128:(k + 1) * 128, :])
            nc.sync.dma_start(cos_sb[:], cosR[:])
            nc.sync.dma_start(sin_sb[:], sinS[:])
            nc.sync.dma_start(mask_sb[:], maskM[:])
            nc.sync.dma_start(id_sb[:], ident[:])
            nc.sync.dma_start(ones_sb[:], onesI[:])
            nc.vector.tensor_copy(onesb_sb[:], ones_sb[:])
            # ones column at col 64 of every v slot
            nc.vector.tensor_copy(
                v_sb.rearrange("p (s d) -> p s d", d=65)[:, :, 64:65],
                onesb_sb[:, 0:64].unsqueeze(2))

            # ================= QKV projection + RoPE =================
            for q in range(NQ):
                t0 = q * QT
                for k in range(8):
                    nc.sync.dma_start(xq_sb[:, k * QT:(k + 1) * QT],
                                      xT[k * 128:(k + 1) * 128, t0:t0 + QT])
                # q,k: out [feat 128, tok 512] for m = QA QB KA KB
                for m in range(4):
                    ps = psA.tile([128, 512], F32, tag="grp")
                    for k in range(8):
                        nc.tensor.matmul(ps[:],
                                         wqk_sb[:, k * 512 + m * 128: k * 512 + (m + 1) * 128],
                                         xq_sb[:, k * QT:(k + 1) * QT],
                                         start=(k == 0), stop=(k == 7))
                    raw = rawp.tile([128, QT], B16, tag="raw")
                    nc.vector.tensor_copy(raw[:], ps[:])
                    swp = swpp.tile([128, QT], B16, tag="swp")
                    nc.sync.dma_start(swp[0:32, :], raw[32:64, :])
                    nc.sync.dma_start(swp[32:64, :], raw[0:32, :])
                    nc.sync.dma_start(swp[64:96, :], raw[96:128, :])
                    nc.sync.dma_start(swp[96:128, :], raw[64:96, :])
                    rot = qk_rot[m]
                    cs = cos_sb[:, t0:t0 + QT]
                    sn = sin_sb[:, t0:t0 + QT]
                    tmp = swpp.tile([128, QT], B16, tag="tmp")
                    nc.vector.tensor_mul(tmp[:], swp[:], sn)
                    nc.vector.tensor_mul(rot[:, t0:t0 + QT], raw[:], cs)
                    nc.vector.tensor_add(rot[:, t0:t0 + QT], rot[:, t0:t0 + QT], tmp[:])
                # v: out [tok 128, vfeat 256] for 4 token blocks in this quarter
                for mt in range(4):
                    kb = q * 4 + mt
                    psv = psO.tile([128, 512], F32, tag="acc")
                    for k in range(8):
                        nc.tensor.matmul(psv[:, 0:256],
                                         xq_sb[:, k * QT + mt * 128: k * QT + (mt + 1) * 128],
                                         wv_sb[:, k * 256:(k + 1) * 256],
                                         start=(k == 0), stop=(k == 7))
                    # scatter 4 heads into v slots (cols 0:64 of slot (kb*4+h))
                    src = psv[:, 0:256].rearrange("p (h d) -> p h d", h=4)
                    base = kb * 4 * 65
                    dst = v_sb[:, base:base + 4 * 65].rearrange("p (h d) -> p h d", d=65)[:, :, 0:64]
                    nc.vector.tensor_copy(dst, src)

            # ================= attention =================
            for qb in range(4):
                for h in range(4):
                    h2 = h % 2
                    Qt = qk_rot[0] if h < 2 else qk_rot[1]
                    Kt = qk_rot[2] if h < 2 else qk_rot[3]
                    live = 4 * (qb + 1)
                    out_ps = psO.tile([65, 512], F32, tag="acc")
                    kbs = list(range(live))
                    groups = [kbs[i:i + 3] for i in range(0, live, 3)]
                    for grp in groups:
                        st = psA.tile([128, 1536], F32, tag="grp")
                        for i, kb in enumerate(grp):
                            diag = kb >= 4 * qb
                            dl = (kb - 4 * qb) * 128 if diag else 0   # live q cols start
                            nc.tensor.matmul(
                                st[:, i * 512:(i + 1) * 512],
                                Kt[64 * h2:64 * h2 + 64, kb * 128:(kb + 1) * 128],
                                Qt[64 * h2:64 * h2 + 64, qb * 512:(qb + 1) * 512],
                                start=True, stop=(not diag))
                            if diag:
                                di = kb - 4 * qb
                                nc.tensor.matmul(
                                    st[:, i * 512 + dl:(i + 1) * 512],
                                    id_sb[:],
                                    mask_sb[:, di * 512 + dl:(di + 1) * 512],
                                    start=False, stop=True)
                        L = len(grp)
                        pt = ptp.tile([128, 1536], B16, tag="pt")
                        nc.scalar.activation(pt[:, 0:L * 512], st[:, 0:L * 512],
                                             AF.Exp, scale=0.125)
                        for i, kb in enumerate(grp):
                            diag = kb >= 4 * qb
                            dl = (kb - 4 * qb) * 128 if diag else 0
                            nc.tensor.matmul(
                                out_ps[:, dl:512],
                                v_sb[:, (kb * 4 + h) * 65:(kb * 4 + h) * 65 + 65],
                                pt[:, i * 512 + dl:(i + 1) * 512],
                                start=(kb == 0), stop=(kb == live - 1))
                    # evacuate + normalize on base-0 tiles, then DMA into AOu
                    stage = stagep.tile([65, 512], F32, tag="stage")
                    nc.vector.tensor_copy(stage[:], out_ps[:])
                    den0 = nrmp.tile([1, 512], F32, tag="den0")
                    nc.sync.dma_start(den0[:], stage[64:65, :])
                    rr = nrmp.tile([1, 512], F32, tag="rr")
                    with nc.allow_low_precision(reason="softmax denominators"):
                        nc.vector.reciprocal_approx_fast(rr[:], den0[:])
                        rrr = nrmp.tile([1, 512], FR, tag="rrr")
                        nc.vector.tensor_copy(rrr[:], rr[:])
                    bc = psO.tile([64, 512], F32, tag="acc")
                    nc.tensor.matmul(bc[:], ones_sb[0:1, 0:64], rrr[:],
                                     start=True, stop=True)
                    nstage = nrmp.tile([64, 512], B16, tag="nstage")
                    nc.vector.tensor_mul(nstage[:], stage[0:64, :], bc[:])
                    AO = aou[0] if h < 2 else aou[1]
                    nc.sync.dma_start(AO[64 * h2:64 * h2 + 64, qb * 512:(qb + 1) * 512],
                                      nstage[:])

            # ================= output projection =================
            for qt in range(16):
                yp = psA.tile([128, 1024], F32, tag="grp")
                for nh in range(2):
                    nc.tensor.matmul(yp[:, nh * 512:(nh + 1) * 512],
                                     aou[0][:, qt * 128:(qt + 1) * 128],
                                     wo_sb[:, 0 * 1024 + nh * 512: 0 * 1024 + (nh + 1) * 512],
                                     start=True, stop=False)
                    nc.tensor.matmul(yp[:, nh * 512:(nh + 1) * 512],
                                     aou[1][:, qt * 128:(qt + 1) * 128],
                                     wo_sb[:, 1 * 1024 + nh * 512: 1 * 1024 + (nh + 1) * 512],
                                     start=False, stop=True)
                ysb = ysbp.tile([128, 1024], F32, tag="y")
                nc.vector.tensor_copy(ysb[:], yp[:])
                nc.sync.dma_start(out[qt * 128:(qt + 1) * 128, :], ysb[:])

    nc.compile()
    return nc


def _core_inputs(x, cos, sin, W_qkv, W_out, core):
    b = core // 4
    hg = core % 4
    heads = list(range(4 * hg, 4 * hg + 4))

    xT = np.ascontiguousarray(x[b].T).astype(BF)
    qrows = np.concatenate([W_qkv[h * 64:(h + 1) * 64] for h in heads], 0)
    krows = np.concatenate([W_qkv[C + h * 64: C + (h + 1) * 64] for h in heads], 0)
    wqk = np.ascontiguousarray(np.concatenate([qrows, krows], 0).T).astype(BF)
    vrows = np.concatenate([W_qkv[2 * C + h * 64: 2 * C + (h + 1) * 64] for h in heads], 0)
    wv = np.ascontiguousarray(vrows.T).astype(BF)
    cols = np.concatenate([np.arange(h * 64, (h + 1) * 64) for h in heads])
    wo = np.ascontiguousarray(W_out[:, cols].T).astype(BF)

    cT = np.ascontiguousarray(cos.T)      # (32, T)
    sT = np.ascontiguousarray(sin.T)
    cosR = np.tile(cT, (4, 1)).astype(BF)
    sinS = np.concatenate([-sT, sT, -sT, sT], 0).astype(BF)

    p = np.arange(128)[:, None]
    j = np.arange(512)[None, :]
    mtiles = [np.where(p + 128 * i <= j, 0.0, NEG).astype(BF) for i in range(4)]
    maskM = np.concatenate(mtiles, 1)

    return {
        "xT": xT, "wqk": wqk, "wv": wv, "wo": wo,
        "cosR": cosR, "sinS": sinS,
        "maskM": np.ascontiguousarray(maskM),
        "ident": np.eye(128).astype(BF),
        "onesI": np.ones((128, 64), dtype=np.float32),
    }


def kernel(x, cos, sin, mask, W_qkv, W_out):
    from concourse import bass_utils

    x = np.asarray(x, dtype=np.float32)
    cos = np.asarray(cos, dtype=np.float32)
    sin = np.asarray(sin, dtype=np.float32)
    W_qkv = np.asarray(W_qkv, dtype=np.float32)
    W_out = np.asarray(W_out, dtype=np.float32)

    if "nc" not in _cache:
        _cache["nc"] = _build()
    nc = _cache["nc"]

    in_maps = [_core_inputs(x, cos, sin, W_qkv, W_out, c) for c in range(NCORES)]
    res = bass_utils.run_bass_kernel_spmd(nc, in_maps, core_ids=list(range(NCORES)))

    y = np.zeros((B, T, C), dtype=np.float32)
    for c in range(NCORES):
        y[c // 4] += res.results[c]["out"]
    return y


# revision 11
# speedup vs baseline: 1.2094x; 1.2094x over previous
"""Fused multi-head causal attention (RoPE) for Trainium2, 8-core SPMD.

Sharding: data-parallel over batch (B=2) x tensor-parallel over heads
(16 heads -> 4 per core, Megatron-style column/row split of the qkv/out
projections). Each core computes a partial (T, C) output; the host sums
the 4 partials per batch element.

Per-core layout:
  - x pre-transposed on host (xT: C x T) so every matmul contraction has
    its contracted dim on SBUF partitions; bf16 operands everywhere on
    the PE (1 cyc/row + fast weight load), fp32 PSUM accumulation.
  - QKV produces q,k transposed [feat, tok] and v natural [tok, feat];
    RoPE runs in [feat, tok] layout, the half-rotation partition swap is
    done by SBUF->SBUF DMA.
  - Scores are computed transposed S^T[k, q] so the softmax sum over k
    becomes a matmul contraction (ones column appended to v); exp runs
    on ScalarE straight out of PSUM with the 1/sqrt(D) folded into the
    activation scale. No max-subtraction (scores are ~N(0,1)). Causal
    masking adds -1e9 on diagonal tiles via an identity matmul, and all
    diagonal-tile matmuls are narrowed to the live column range.
"""

import sys
import numpy as np

if '/opt/trn_rl_repo' not in sys.path:
    sys.path.insert(0, '/opt/trn_rl_repo')

import ml_dtypes

B, T, C, H, D = 2, 2048, 1024, 16, 64
HPC = 4            # heads per core
NCORES = 8
NEG = -1.0e9
BF = ml_dtypes.bfloat16

_cache = {}


def _build():
    import concourse.mybir as mybir
    from concourse import bacc
    import concourse.tile as tile

    F32 = mybir.dt.float32
    FR = mybir.dt.float32r
    B16 = mybir.dt.bfloat16
    AF = mybir.ActivationFunctionType

    nc = bacc.Bacc("TRN2", debug=False, enable_asserts=True)
    xT = nc.dram_tensor("xT", [C, T], B16, kind="ExternalInput").ap()
    wqk = nc.dram_tensor("wqk", [C, 512], B16, kind="ExternalInput").ap()
    wv = nc.dram_tensor("wv", [C, 256], B16, kind="ExternalInput").ap()
    wo = nc.dram_tensor("wo", [256, 1024], B16, kind="ExternalInput").ap()
    cosR = nc.dram_tensor("cosR", [128, T], B16, kind="ExternalInput").ap()
    sinS = nc.dram_tensor("sinS", [128, T], B16, kind="ExternalInput").ap()
    maskM = nc.dram_tensor("maskM", [128, 2048], B16, kind="ExternalInput").ap()
    ident = nc.dram_tensor("ident", [128, 128], B16, kind="ExternalInput").ap()
    onesI = nc.dram_tensor("onesI", [128, 64], FR, kind="ExternalInput").ap()
    out = nc.dram_tensor("out", [T, C], F32, kind="ExternalOutput").ap()

    NQ = 4            # token quarters for streaming x
    QT = T // NQ      # 512 tokens per quarter

    with tile.TileContext(nc) as tc:
        with tc.tile_pool(name="persist", bufs=1) as pp, \
             tc.tile_pool(name="rawp", bufs=2) as rawp, \
             tc.tile_pool(name="swpp", bufs=2) as swpp, \
             tc.tile_pool(name="ptp", bufs=3) as ptp, \
             tc.tile_pool(name="stagep", bufs=2) as stagep, \
             tc.tile_pool(name="nrmp", bufs=2) as nrmp, \
             tc.tile_pool(name="ysbp", bufs=2) as ysbp, \
             tc.tile_pool(name="psA", bufs=3, space="PSUM") as psA, \
             tc.tile_pool(name="psO", bufs=2, space="PSUM") as psO:

            # ---- persistent SBUF tensors ----
            wqk_sb = pp.tile([128, 8 * 512], B16, tag="wqk")
            wv_sb = pp.tile([128, 8 * 256], B16, tag="wv")
            wo_sb = pp.tile([128, 2 * 1024], B16, tag="wo")
            cos_sb = pp.tile([128, T], B16, tag="cos")
            sin_sb = pp.tile([128, T], B16, tag="sin")
            mask_sb = pp.tile([128, 2048], B16, tag="mask")
            id_sb = pp.tile([128, 128], B16, tag="id")
            ones_sb = pp.tile([128, 64], FR, tag="ones")
            onesb_sb = pp.tile([128, 64], B16, tag="onesb")
            qk_rot = [pp.tile([128, T], B16, tag=f"rot{i}", name=f"rot{i}") for i in range(4)]
            v_sb = pp.tile([128, 16 * HPC * 65], B16, tag="v")
            aou = [pp.tile([128, T], B16, tag=f"aou{i}", name=f"aou{i}") for i in range(2)]
            xq_sb = pp.tile([128, 8 * QT], B16, tag="xq")  # one token-quarter of xT

            for k in range(8):
                nc.sync.dma_start(wqk_sb[:, k * 512:(k + 1) * 512], wqk[k * 128:(k + 1) * 128, :])
                nc.sync.dma_start(wv_sb[:, k * 256:(k + 1) * 256], wv[k * 128:(k + 1) * 128, :])
            for k in range(2):
                nc.sync.dma_start(wo_sb[:, k * 1024:(k + 1) * 1024], wo[k * 128:(k + 1) * 128, :])
            nc.sync.dma_start(cos_sb[:], cosR[:])
            nc.sync.dma_start(sin_sb[:], sinS[:])
            nc.sync.dma_start(mask_sb[:], maskM[:])
            nc.sync.dma_start(id_sb[:], ident[:])
            nc.sync.dma_start(ones_sb[:], onesI[:])
            nc.vector.tensor_copy(onesb_sb[:], ones_sb[:])
            # ones column at col 64 of every v slot
            nc.vector.tensor_copy(
                v_sb.rearrange("p (s d) -> p s d", d=65)[:, :, 64:65],
                onesb_sb[:, 0:64].unsqueeze(2))

            # ================= QKV projection + RoPE =================
            for q in range(NQ):
                t0 = q * QT
                for k in range(8):
                    nc.sync.dma_start(xq_sb[:, k * QT:(k + 1) * QT],
                                      xT[k * 128:(k + 1) * 128, t0:t0 + QT])
                # q,k: out [feat 128, tok 512] for m = QA QB KA KB
                for m in range(4):
                    ps = psA.tile([128, 512], F32, tag="grp")
                    for k in range(8):
                        nc.tensor.matmul(ps[:],
                                         wqk_sb[:, k * 512 + m * 128: k * 512 + (m + 1) * 128],
                                         xq_sb[:, k * QT:(k + 1) * QT],
                                         start=(k == 0), stop=(k == 7))
                    raw = rawp.tile([128, QT], B16, tag="raw")
                    nc.vector.tensor_copy(raw[:], ps[:])
                    swp = swpp.tile([128, QT], B16, tag="swp")
                    nc.sync.dma_start(swp[0:32, :], raw[32:64, :])
                    nc.sync.dma_start(swp[32:64, :], raw[0:32, :])
                    nc.sync.dma_start(swp[64:96, :], raw[96:128, :])
                    nc.sync.dma_start(swp[96:128, :], raw[64:96, :])
                    rot = qk_rot[m]
                    cs = cos_sb[:, t0:t0 + QT]
                    sn = sin_sb[:, t0:t0 + QT]
                    tmp = swpp.tile([128, QT], B16, tag="tmp")
                    nc.vector.tensor_mul(tmp[:], swp[:], sn)
                    nc.vector.tensor_mul(rot[:, t0:t0 + QT], raw[:], cs)
                    nc.vector.tensor_add(rot[:, t0:t0 + QT], rot[:, t0:t0 + QT], tmp[:])
                # v: out [tok 128, vfeat 256] for 4 token blocks in this quarter
                for mt in range(4):
                    kb = q * 4 + mt
                    psv = psO.tile([128, 512], F32, tag="acc")
                    for k in range(8):
                        nc.tensor.matmul(psv[:, 0:256],
                                         xq_sb[:, k * QT + mt * 128: k * QT + (mt + 1) * 128],
                                         wv_sb[:, k * 256:(k + 1) * 256],
                                         start=(k == 0), stop=(k == 7))
                    # scatter 4 heads into v slots (cols 0:64 of slot (kb*4+h))
                    src = psv[:, 0:256].rearrange("p (h d) -> p h d", h=4)
                    base = kb * 4 * 65
                    dst = v_sb[:, base:base + 4 * 65].rearrange("p (h d) -> p h d", d=65)[:, :, 0:64]
                    nc.vector.tensor_copy(dst, src)

            # ================= attention =================
            # Per (qb, head-pair): software-pipelined groups of 2 k-blocks.
            # Emission order interleaves both heads' S^T for group g with
            # their PV for group g-1, so the PE never sits behind an exp.
            def st_group(h, qb, g, st):
                h2 = h % 2
                Qt = qk_rot[0] if h < 2 else qk_rot[1]
                Kt = qk_rot[2] if h < 2 else qk_rot[3]
                for i in range(2):
                    kb = 2 * g + i
                    diag = kb >= 4 * qb
                    nc.tensor.matmul(
                        st[:, i * 512:(i + 1) * 512],
                        Kt[64 * h2:64 * h2 + 64, kb * 128:(kb + 1) * 128],
                        Qt[64 * h2:64 * h2 + 64, qb * 512:(qb + 1) * 512],
                        start=True, stop=(not diag))
                for i in range(2):
                    kb = 2 * g + i
                    if kb >= 4 * qb:
                        di = kb - 4 * qb
                        dl = di * 128
                        nc.tensor.matmul(
                            st[:, i * 512 + dl:(i + 1) * 512],
                            id_sb[:],
                            mask_sb[:, di * 512 + dl:(di + 1) * 512],
                            start=False, stop=True)

            def pv_group(h, qb, g, pt, out_ps, live):
                for i in range(2):
                    kb = 2 * g + i
                    diag = kb >= 4 * qb
                    dl = (kb - 4 * qb) * 128 if diag else 0
                    nc.tensor.matmul(
                        out_ps[:, dl:512],
                        v_sb[:, (kb * 4 + h) * 65:(kb * 4 + h) * 65 + 65],
                        pt[:, i * 512 + dl:(i + 1) * 512],
                        start=(kb == 0), stop=(kb == live - 1))

            for qb in range(4):
                live = 4 * (qb + 1)
                ng = live // 2
                for hp in ((0, 1), (2, 3)):
                    out_ps = {h: psO.tile([65, 512], F32, tag="acc",
                                          name=f"ops{qb}_{h}") for h in hp}
                    pts = {}
                    for g in range(ng + 1):
                        for h in hp:
                            if g < ng:
                                st = psA.tile([128, 1024], F32, tag="grp",
                                              name=f"st{qb}_{h}_{g}")
                                st_group(h, qb, g, st)
                                pt = ptp.tile([128, 1024], B16, tag="pt",
                                              name=f"pt{qb}_{h}_{g}")
                                nc.scalar.activation(pt[:], st[:], AF.Exp, scale=0.125)
                                pts[h] = pts.get(h, {})
                                pts[h][g] = pt
                        for h in hp:
                            if g >= 1:
                                pv_group(h, qb, g - 1, pts[h].pop(g - 1),
                                         out_ps[h], live)
                    # evacuate + normalize on base-0 tiles, then DMA into AOu
                    for h in hp:
                        h2 = h % 2
                        stage = stagep.tile([65, 512], F32, tag="stage",
                                            name=f"stage{qb}_{h}")
                        nc.vector.tensor_copy(stage[:], out_ps[h][:])
                        den0 = nrmp.tile([1, 512], F32, tag="den0")
                        nc.sync.dma_start(den0[:], stage[64:65, :])
                        rr = nrmp.tile([1, 512], F32, tag="rr")
                        with nc.allow_low_precision(reason="softmax denominators"):
                            nc.vector.reciprocal_approx_fast(rr[:], den0[:])
                            rrr = nrmp.tile([1, 512], FR, tag="rrr")
                            nc.vector.tensor_copy(rrr[:], rr[:])
                        bc = psO.tile([64, 512], F32, tag="acc")
                        nc.tensor.matmul(bc[:], ones_sb[0:1, 0:64], rrr[:],
                                         start=True, stop=True)
                        nstage = nrmp.tile([64, 512], B16, tag="nstage")
                        nc.vector.tensor_mul(nstage[:], stage[0:64, :], bc[:])
                        AO = aou[0] if h < 2 else aou[1]
                        nc.sync.dma_start(
                            AO[64 * h2:64 * h2 + 64, qb * 512:(qb + 1) * 512],
                            nstage[:])

            # ================= output projection =================
            for qt in range(16):
                yp = psA.tile([128, 1024], F32, tag="grp")
                for nh in range(2):
                    nc.tensor.matmul(yp[:, nh * 512:(nh + 1) * 512],
                                     aou[0][:, qt * 128:(qt + 1) * 128],
                                     wo_sb[:, 0 * 1024 + nh * 512: 0 * 1024 + (nh + 1) * 512],
                                     start=True, stop=False)
                    nc.tensor.matmul(yp[:, nh * 512:(nh + 1) * 512],
                                     aou[1][:, qt * 128:(qt + 1) * 128],
                                     wo_sb[:, 1 * 1024 + nh * 512: 1 * 1024 + (nh + 1) * 512],
                                     start=False, stop=True)
                ysb = ysbp.tile([128, 1024], F32, tag="y")
                nc.vector.tensor_copy(ysb[:], yp[:])
                nc.sync.dma_start(out[qt * 128:(qt + 1) * 128, :], ysb[:])

    nc.compile()
    return nc


def _core_inputs(x, cos, sin, W_qkv, W_out, core):
    b = core // 4
    hg = core % 4
    heads = list(range(4 * hg, 4 * hg + 4))

    xT = np.ascontiguousarray(x[b].T).astype(BF)
    qrows = np.concatenate([W_qkv[h * 64:(h + 1) * 64] for h in heads], 0)
    krows = np.concatenate([W_qkv[C + h * 64: C + (h + 1) * 64] for h in heads], 0)
    wqk = np.ascontiguousarray(np.concatenate([qrows, krows], 0).T).astype(BF)
    vrows = np.concatenate([W_qkv[2 * C + h * 64: 2 * C + (h + 1) * 64] for h in heads], 0)
    wv = np.ascontiguousarray(vrows.T).astype(BF)
    cols = np.concatenate([np.arange(h * 64, (h + 1) * 64) for h in heads])
    wo = np.ascontiguousarray(W_out[:, cols].T).astype(BF)

    cT = np.ascontiguousarray(cos.T)      # (32, T)
    sT = np.ascontiguousarray(sin.T)
    cosR = np.tile(cT, (4, 1)).astype(BF)
    sinS = np.concatenate([-sT, sT, -sT, sT], 0).astype(BF)

    p = np.arange(128)[:, None]
    j = np.arange(512)[None, :]
    mtiles = [np.where(p + 128 * i <= j, 0.0, NEG).astype(BF) for i in range(4)]
    maskM = np.concatenate(mtiles, 1)

    return {
        "xT": xT, "wqk": wqk, "wv": wv, "wo": wo,
        "cosR": cosR, "sinS": sinS,
        "maskM": np.ascontiguousarray(maskM),
        "ident": np.eye(128).astype(BF),
        "onesI": np.ones((128, 64), dtype=np.float32),
    }


def kernel(x, cos, sin, mask, W_qkv, W_out):
    from concourse import bass_utils

    x = np.asarray(x, dtype=np.float32)
    cos = np.asarray(cos, dtype=np.float32)
    sin = np.asarray(sin, dtype=np.float32)
    W_qkv = np.asarray(W_qkv, dtype=np.float32)
    W_out = np.asarray(W_out, dtype=np.float32)

    if "nc" not in _cache:
        _cache["nc"] = _build()
    nc = _cache["nc"]

    in_maps = [_core_inputs(x, cos, sin, W_qkv, W_out, c) for c in range(NCORES)]
    res = bass_utils.run_bass_kernel_spmd(nc, in_maps, core_ids=list(range(NCORES)))

    y = np.zeros((B, T, C), dtype=np.float32)
    for c in range(NCORES):
        y[c // 4] += res.results[c]["out"]
    return y
